# revision 26
# baseline (speedup 1.0000x reference)
"""Trainium2 kernel for nn_ACS_8942121910988 (topk_masking).

The reference network is affine in x: four conv/BN branches -> concat
-> top-k channel gather with sigmoid scaling.  Everything folds into a
single 3x3 convolution (64 -> 64 channels) plus a bias field that takes
9 distinct per-channel values by border region (interior / edges /
corners) because the inner-branch BN biases leak through the zero-padded
outer convs.

Per-core layout (1 batch image per core, 8 cores):
  - x is DMA'd into SBUF stripes with "parity packing": slot t holds
    x row (r0+2t-1) channels in partitions 0-63 and row (r0+2t) in
    partitions 64-127.  Each x row lands in SBUF exactly once.
  - An output row-pair (h, h+1) is computed with 6 matmuls (2 row-phases
    x 3 kx taps), each [K=128] x [M=128] x [N=256] in float32r
    (1 cycle/row at N>=256), accumulating in PSUM.  The lhsT matrices
    carry zero blocks that implement the ky tap structure.
  - ScalarE evacuates PSUM -> SBUF adding the per-channel interior bias;
    VectorE patches the w=0 / w=255 columns and the 4 corners.
"""

import sys

if "/opt/trn_rl_repo" not in sys.path:
    sys.path.insert(0, "/opt/trn_rl_repo")

import numpy as np

BN_EPS = 1e-5
B, C, H, W = 8, 64, 256, 256
NCORES = 8
R = 16            # output rows per stripe
S = R // 2        # row-pairs per stripe
NSTRIPES = H // R
SLOTW = W + 2     # slot width incl. 1 zero pad column each side

# pass order: (row-phase, kx). Pass 0 must be full-width (it is: all are).
PASSES = [("A", 1), ("A", 0), ("A", 2), ("B", 0), ("B", 1), ("B", 2)]


def fold_params(w_main, bn_main, w_1x1, bn_1x1, w_3x3_1, bn_3x3_1,
                w_3x3_2, bn_3x3_2, w_avg, bn_avg_1, bn_avg_2, c_score):
    """Fold all branches + BN + top-k gather into (W_eff, TB).

    W_eff: [64, 64, 3, 3] conv weight, out = conv3x3(x, W_eff, pad=1) + bias
    TB:    [64, 3, 3] per-tap bias table; bias at (h, w) is the sum of
           TB over taps whose input pixel is inside the image.
    """
    f8 = np.float64
    def inv_beta(p):
        p = np.asarray(p, f8)
        inv = p[0] / np.sqrt(p[3] + BN_EPS)
        return inv, p[1] - p[2] * inv

    W_all = np.zeros((2 * C, C, 3, 3), f8)
    TB_all = np.zeros((2 * C, 3, 3), f8)

    # branch_main: conv3x3 + BN
    inv, beta = inv_beta(bn_main)
    W_all[0:32] = np.asarray(w_main, f8) * inv[:, None, None, None]
    TB_all[0:32, 1, 1] += beta

    # branch_1x1: conv1x1 + BN (center tap only)
    inv, beta = inv_beta(bn_1x1)
    W_all[32:64, :, 1, 1] = np.asarray(w_1x1, f8)[:, :, 0, 0] * inv[:, None]
    TB_all[32:64, 1, 1] += beta

    # branch_3x3: BN2(conv3x3(BN1(conv1x1(x))))
    inv_a, ba = inv_beta(bn_3x3_1)
    A = np.asarray(w_3x3_1, f8)[:, :, 0, 0] * inv_a[:, None]        # [32, 64]
    inv2, b2 = inv_beta(bn_3x3_2)
    W2 = np.asarray(w_3x3_2, f8)                                    # [32,32,3,3]
    W_all[64:96] = np.einsum("omyx,mc->ocyx", W2, A) * inv2[:, None, None, None]
    TB_all[64:96] = np.einsum("omyx,m->oyx", W2, ba) * inv2[:, None, None]
    TB_all[64:96, 1, 1] += b2

    # branch_avg: BN2(avgpool3(BN1(conv1x1(x)))); avgpool = uniform 1/9 taps
    inv_v1, bv1 = inv_beta(bn_avg_1)
    V = np.asarray(w_avg, f8)[:, :, 0, 0] * inv_v1[:, None]         # [32, 64]
    inv_v2, bv2 = inv_beta(bn_avg_2)
    W_all[96:128] = (V * inv_v2[:, None] / 9.0)[:, :, None, None] * np.ones((1, 1, 3, 3))
    TB_all[96:128] = np.broadcast_to((inv_v2 * bv1 / 9.0)[:, None, None], (32, 3, 3)).copy()
    TB_all[96:128, 1, 1] += bv2

    # top-k channel gather + sigmoid scale
    cs = np.asarray(c_score, f8)
    ind = np.argsort(-cs, kind="stable")[:C]
    scale = 1.0 / (1.0 + np.exp(-cs[ind]))
    W_eff = (W_all[ind] * scale[:, None, None, None]).astype(np.float32)
    TB = (TB_all[ind] * scale[:, None, None]).astype(np.float32)
    return W_eff, TB


def build_device_tensors(W_eff, TB):
    """Pack lhsT weight matrices and bias vectors for the device."""
    Wt = W_eff.transpose(1, 0, 2, 3)          # [c, o, ky, kx]
    wpack = np.zeros((6, 128, 128), np.float32)
    for widx, (ab, kx) in enumerate(PASSES):
        m = wpack[widx]
        if ab == "A":
            # rhs slot t = rows (2t-1, 2t); out pair (2t, 2t+1)
            m[0:64, 0:64] = Wt[:, :, 0, kx]     # x row 2t-1 -> out 2t (ky=0)
            m[64:128, 0:64] = Wt[:, :, 1, kx]   # x row 2t   -> out 2t (ky=1)
            m[64:128, 64:128] = Wt[:, :, 0, kx] # x row 2t   -> out 2t+1 (ky=0)
        else:
            # rhs slot t+1 = rows (2t+1, 2t+2)
            m[0:64, 0:64] = Wt[:, :, 2, kx]     # x row 2t+1 -> out 2t (ky=2)
            m[0:64, 64:128] = Wt[:, :, 1, kx]   # x row 2t+1 -> out 2t+1 (ky=1)
            m[64:128, 64:128] = Wt[:, :, 2, kx] # x row 2t+2 -> out 2t+1 (ky=2)
    import ml_dtypes
    wp = np.ascontiguousarray(wpack.transpose(1, 0, 2)).reshape(128, 6 * 128)
    wp = wp.astype(ml_dtypes.bfloat16)

    bias_int = TB.sum(axis=(1, 2))            # all 9 taps valid
    ctop = -TB[:, 0, :].sum(axis=1)           # row 0: ky=0 taps invalid
    cbot = -TB[:, 2, :].sum(axis=1)
    cleft = -TB[:, :, 0].sum(axis=1)
    cright = -TB[:, :, 2].sum(axis=1)

    bv = np.zeros((128, 9), np.float32)
    bv[0:64, 0] = bias_int;         bv[64:128, 0] = bias_int
    bv[0:64, 1] = bias_int + ctop;  bv[64:128, 1] = bias_int
    bv[0:64, 2] = bias_int;         bv[64:128, 2] = bias_int + cbot
    bv[0:64, 3] = cleft;            bv[64:128, 3] = cleft
    bv[0:64, 4] = cright;           bv[64:128, 4] = cright
    bv[0:64, 5] = TB[:, 0, 0]       # TL corner (+TB[0,0] by inclusion-exclusion)
    bv[0:64, 6] = TB[:, 0, 2]       # TR
    bv[64:128, 7] = TB[:, 2, 0]     # BL
    bv[64:128, 8] = TB[:, 2, 2]     # BR
    return wp, bv


def pack_x(img):
    """Host-side repack of one [64, 256, 256] image into the parity-slot
    layout [129, 128, 258]: slot g holds x rows (2g-1, 2g) — channels of
    row 2g-1 at p=0..63, row 2g at p=64..127 — with 1 zero pad column on
    each side.  Rows -1 and 256 are zeros."""
    xh = np.zeros((129, 2, 64, SLOTW), np.float32)
    # xh[g, 0, c, 1:257] = x[c, 2g-1, :]  (g = 1..128)
    xh[1:, 0, :, 1:W + 1] = img[:, 1::2, :].transpose(1, 0, 2)
    # xh[g, 1, c, 1:257] = x[c, 2g, :]    (g = 0..127)
    xh[:128, 1, :, 1:W + 1] = img[:, 0::2, :].transpose(1, 0, 2)
    import ml_dtypes
    return np.ascontiguousarray(xh.reshape(129, 128, SLOTW)).astype(ml_dtypes.bfloat16)


def unpack_out(od):
    """Inverse of the device output layout [128 pairs, 128 (i c), 256]."""
    return np.ascontiguousarray(
        od.reshape(128, 2, 64, W).transpose(2, 0, 1, 3).reshape(C, H, W)).astype(np.float32)


def build_bass():
    import concourse.bass as bass
    import concourse.bacc as bacc
    import concourse.mybir as mybir
    import concourse.tile as tile

    f32 = mybir.dt.float32
    bf16 = mybir.dt.bfloat16

    nc = bacc.Bacc()
    x = nc.dram_tensor("x", [129, 128, SLOTW], bf16, kind="ExternalInput")
    wp = nc.dram_tensor("wp", [128, 6 * 128], bf16, kind="ExternalInput")
    bvt = nc.dram_tensor("bv", [128, 9], f32, kind="ExternalInput")
    out = nc.dram_tensor("out", [H // 2, 128, W], bf16, kind="ExternalOutput")

    with tile.TileContext(nc) as tc:
        with (
            tc.tile_pool(name="const", bufs=1) as cpool,
            tc.tile_pool(name="xin", bufs=NSTRIPES) as xpool,
            tc.tile_pool(name="oout", bufs=NSTRIPES) as opool,
            tc.tile_pool(name="acc", bufs=8, space="PSUM") as ppool,
        ):
            wt = cpool.tile([128, 6, 128], bf16)
            nc.sync.dma_start(out=wt, in_=wp.rearrange("k (w m) -> k w m", w=6))
            bt = cpool.tile([128, 9], f32)
            nc.sync.dma_start(out=bt, in_=bvt[:, :])

            def absorb(engine, aps):
                """Sequencer NoOp reading `aps`: it alone carries the sem
                waits (this walrus allows 1 sync wait per engine/DMA
                instruction; sequencer NoOps may carry several), and later
                instructions on `engine` inherit the observed clock."""
                noop = mybir.InstNoOp(
                    name=nc.get_next_instruction_name(),
                    engine=engine,
                    bass_nofuse=True,
                    ins=[e.lower_ap(ap) for e, ap in aps],
                )
                nc.add_instruction(noop)

            PE, SP, DVE = (mybir.EngineType.PE, mybir.EngineType.SP,
                           mybir.EngineType.DVE)
            # absorb the wt/bt DMA waits once up front
            absorb(PE, [(nc.tensor, wt[:, :, :])])
            absorb(DVE, [(nc.vector, bt[:, :])])

            ot_hist = []
            for s in range(NSTRIPES):
                p0 = s * S  # global row-pair index of this stripe's first pair
                xt = xpool.tile([128, S + 1, SLOTW], bf16)
                # one 128-partition DMA per stripe: 9 pair-slots incl. halo
                nc.sync.dma_start(
                    out=xt,
                    in_=x[p0:p0 + S + 1].rearrange("g p w -> p g w"))

                # absorb the x-DMA wait and (via a read of the stripe-s-2
                # output tile, whose evacuations freed the PSUM banks this
                # stripe reuses) the DVE wait, before any matmul issues
                pe_aps = [(nc.tensor, xt[:, :, :])]
                if len(ot_hist) >= 2:
                    pe_aps.append((nc.tensor, ot_hist[-2][:, :, :]))
                absorb(PE, pe_aps)

                # N=512 matmuls: each covers two row-pairs (a "quad"); the
                # rhs is a strided [128, 2, 256] AP over adjacent slots.
                NQ = S // 2
                pt = [ppool.tile([128, 2, W], f32, name="pt", tag="pt")
                      for _ in range(NQ)]
                for widx, (ab, kx) in enumerate(PASSES):
                    for q in range(NQ):
                        slot = 2 * q if ab == "A" else 2 * q + 1
                        nc.tensor.matmul(
                            pt[q][:, :, :],
                            lhsT=wt[:, widx, :],
                            rhs=xt[:, slot:slot + 2, kx:kx + W],
                            start=(widx == 0),
                            stop=(widx == len(PASSES) - 1),
                        )

                # bufs=NSTRIPES: ot slots are never recycled, so evacuations
                # never carry a WAR wait on an out-DMA (1-wait budget)
                ot = opool.tile([128, S, W], bf16)
                for q in range(NQ):
                    if s == 0 and q == 0:
                        # first pair of the image needs the top-row bias
                        nc.vector.tensor_scalar_add(ot[:, 0, :], pt[q][:, 0, :],
                                                    bt[:, 1:2])
                        nc.vector.tensor_scalar_add(ot[:, 1, :], pt[q][:, 1, :],
                                                    bt[:, 0:1])
                    elif s == NSTRIPES - 1 and q == NQ - 1:
                        nc.vector.tensor_scalar_add(ot[:, S - 2, :],
                                                    pt[q][:, 0, :], bt[:, 0:1])
                        nc.vector.tensor_scalar_add(ot[:, S - 1, :],
                                                    pt[q][:, 1, :], bt[:, 2:3])
                    else:
                        nc.vector.tensor_scalar_add(
                            ot[:, 2 * q:2 * q + 2, :], pt[q][:, :, :],
                            bt[:, 0:1])

                # column corrections for the bias-through-padding terms
                nc.vector.tensor_scalar_add(ot[:, :, 0:1], ot[:, :, 0:1],
                                            bt[:, 3:4])
                nc.vector.tensor_scalar_add(ot[:, :, W - 1:W], ot[:, :, W - 1:W],
                                            bt[:, 4:5])
                if s == 0:
                    nc.vector.tensor_scalar_add(ot[0:64, 0, 0:1],
                                                ot[0:64, 0, 0:1], bt[0:64, 5:6])
                    nc.vector.tensor_scalar_add(ot[0:64, 0, W - 1:W],
                                                ot[0:64, 0, W - 1:W], bt[0:64, 6:7])
                if s == NSTRIPES - 1:
                    nc.vector.tensor_scalar_add(ot[64:128, S - 1, 0:1],
                                                ot[64:128, S - 1, 0:1],
                                                bt[64:128, 7:8])
                    nc.vector.tensor_scalar_add(ot[64:128, S - 1, W - 1:W],
                                                ot[64:128, S - 1, W - 1:W],
                                                bt[64:128, 8:9])

                absorb(SP, [(nc.sync, ot[:, :, :])])
                nc.sync.dma_start(
                    out=out[p0:p0 + S].rearrange("g p w -> p g w"), in_=ot)
                ot_hist.append(ot)

    nc.compile()
    return nc


_BASS_CACHE = {}


def run(inputs, trace=False, tmpdir=None):
    from concourse.bass_utils import run_bass_kernel_spmd

    W_eff, TB = fold_params(
        inputs["w_main"], inputs["bn_main"], inputs["w_1x1"], inputs["bn_1x1"],
        inputs["w_3x3_1"], inputs["bn_3x3_1"], inputs["w_3x3_2"],
        inputs["bn_3x3_2"], inputs["w_avg"], inputs["bn_avg_1"],
        inputs["bn_avg_2"], inputs["c_score"])
    wp, bv = build_device_tensors(W_eff, TB)

    if "nc" not in _BASS_CACHE:
        _BASS_CACHE["nc"] = build_bass()
    nc = _BASS_CACHE["nc"]

    x = np.asarray(inputs["x"], np.float32)
    in_maps = [{"x": pack_x(x[i]), "wp": wp, "bv": bv} for i in range(NCORES)]
    res = run_bass_kernel_spmd(nc, in_maps, core_ids=list(range(NCORES)),
                               trace=trace, tmpdir=tmpdir)
    outs = np.stack([unpack_out(np.asarray(r["out"])) for r in res.results])
    return outs, res


def kernel(**inputs) -> np.ndarray:
    outs, _ = run(inputs, trace=False)
    return outs


# revision 27
# speedup vs baseline: 1.0555x; 1.0555x over previous
"""Trainium2 kernel for nn_ACS_8942121910988 (topk_masking).

The reference network is affine in x: four conv/BN branches -> concat
-> top-k channel gather with sigmoid scaling.  Everything folds into a
single 3x3 convolution (64 -> 64 channels) plus a bias field that takes
9 distinct per-channel values by border region (interior / edges /
corners) because the inner-branch BN biases leak through the zero-padded
outer convs.

Per-core layout (1 batch image per core, 8 cores):
  - x is DMA'd into SBUF stripes with "parity packing": slot t holds
    x row (r0+2t-1) channels in partitions 0-63 and row (r0+2t) in
    partitions 64-127.  Each x row lands in SBUF exactly once.
  - An output row-pair (h, h+1) is computed with 6 matmuls (2 row-phases
    x 3 kx taps), each [K=128] x [M=128] x [N=256] in float32r
    (1 cycle/row at N>=256), accumulating in PSUM.  The lhsT matrices
    carry zero blocks that implement the ky tap structure.
  - ScalarE evacuates PSUM -> SBUF adding the per-channel interior bias;
    VectorE patches the w=0 / w=255 columns and the 4 corners.
"""

import sys

if "/opt/trn_rl_repo" not in sys.path:
    sys.path.insert(0, "/opt/trn_rl_repo")

import numpy as np

BN_EPS = 1e-5
B, C, H, W = 8, 64, 256, 256
NCORES = 8
R = 16            # output rows per stripe
S = R // 2        # row-pairs per stripe
NSTRIPES = H // R
# small edge stripes let the PE start sooner and drain earlier
STRIPES = [2, 2, 4] + [8] * 14 + [4, 2, 2]
assert sum(STRIPES) == H // 2
SLOTW = W + 2     # slot width incl. 1 zero pad column each side

# pass order: (row-phase, kx). Pass 0 must be full-width (it is: all are).
PASSES = [("A", 1), ("A", 0), ("A", 2), ("B", 0), ("B", 1), ("B", 2)]


def fold_params(w_main, bn_main, w_1x1, bn_1x1, w_3x3_1, bn_3x3_1,
                w_3x3_2, bn_3x3_2, w_avg, bn_avg_1, bn_avg_2, c_score):
    """Fold all branches + BN + top-k gather into (W_eff, TB).

    W_eff: [64, 64, 3, 3] conv weight, out = conv3x3(x, W_eff, pad=1) + bias
    TB:    [64, 3, 3] per-tap bias table; bias at (h, w) is the sum of
           TB over taps whose input pixel is inside the image.
    """
    f8 = np.float64
    def inv_beta(p):
        p = np.asarray(p, f8)
        inv = p[0] / np.sqrt(p[3] + BN_EPS)
        return inv, p[1] - p[2] * inv

    W_all = np.zeros((2 * C, C, 3, 3), f8)
    TB_all = np.zeros((2 * C, 3, 3), f8)

    # branch_main: conv3x3 + BN
    inv, beta = inv_beta(bn_main)
    W_all[0:32] = np.asarray(w_main, f8) * inv[:, None, None, None]
    TB_all[0:32, 1, 1] += beta

    # branch_1x1: conv1x1 + BN (center tap only)
    inv, beta = inv_beta(bn_1x1)
    W_all[32:64, :, 1, 1] = np.asarray(w_1x1, f8)[:, :, 0, 0] * inv[:, None]
    TB_all[32:64, 1, 1] += beta

    # branch_3x3: BN2(conv3x3(BN1(conv1x1(x))))
    inv_a, ba = inv_beta(bn_3x3_1)
    A = np.asarray(w_3x3_1, f8)[:, :, 0, 0] * inv_a[:, None]        # [32, 64]
    inv2, b2 = inv_beta(bn_3x3_2)
    W2 = np.asarray(w_3x3_2, f8)                                    # [32,32,3,3]
    W_all[64:96] = np.einsum("omyx,mc->ocyx", W2, A) * inv2[:, None, None, None]
    TB_all[64:96] = np.einsum("omyx,m->oyx", W2, ba) * inv2[:, None, None]
    TB_all[64:96, 1, 1] += b2

    # branch_avg: BN2(avgpool3(BN1(conv1x1(x)))); avgpool = uniform 1/9 taps
    inv_v1, bv1 = inv_beta(bn_avg_1)
    V = np.asarray(w_avg, f8)[:, :, 0, 0] * inv_v1[:, None]         # [32, 64]
    inv_v2, bv2 = inv_beta(bn_avg_2)
    W_all[96:128] = (V * inv_v2[:, None] / 9.0)[:, :, None, None] * np.ones((1, 1, 3, 3))
    TB_all[96:128] = np.broadcast_to((inv_v2 * bv1 / 9.0)[:, None, None], (32, 3, 3)).copy()
    TB_all[96:128, 1, 1] += bv2

    # top-k channel gather + sigmoid scale
    cs = np.asarray(c_score, f8)
    ind = np.argsort(-cs, kind="stable")[:C]
    scale = 1.0 / (1.0 + np.exp(-cs[ind]))
    W_eff = (W_all[ind] * scale[:, None, None, None]).astype(np.float32)
    TB = (TB_all[ind] * scale[:, None, None]).astype(np.float32)
    return W_eff, TB


def build_device_tensors(W_eff, TB):
    """Pack lhsT weight matrices and bias vectors for the device."""
    Wt = W_eff.transpose(1, 0, 2, 3)          # [c, o, ky, kx]
    wpack = np.zeros((6, 128, 128), np.float32)
    for widx, (ab, kx) in enumerate(PASSES):
        m = wpack[widx]
        if ab == "A":
            # rhs slot t = rows (2t-1, 2t); out pair (2t, 2t+1)
            m[0:64, 0:64] = Wt[:, :, 0, kx]     # x row 2t-1 -> out 2t (ky=0)
            m[64:128, 0:64] = Wt[:, :, 1, kx]   # x row 2t   -> out 2t (ky=1)
            m[64:128, 64:128] = Wt[:, :, 0, kx] # x row 2t   -> out 2t+1 (ky=0)
        else:
            # rhs slot t+1 = rows (2t+1, 2t+2)
            m[0:64, 0:64] = Wt[:, :, 2, kx]     # x row 2t+1 -> out 2t (ky=2)
            m[0:64, 64:128] = Wt[:, :, 1, kx]   # x row 2t+1 -> out 2t+1 (ky=1)
            m[64:128, 64:128] = Wt[:, :, 2, kx] # x row 2t+2 -> out 2t+1 (ky=2)
    import ml_dtypes
    wp = np.ascontiguousarray(wpack.transpose(1, 0, 2)).reshape(128, 6 * 128)
    wp = wp.astype(ml_dtypes.bfloat16)

    bias_int = TB.sum(axis=(1, 2))            # all 9 taps valid
    ctop = -TB[:, 0, :].sum(axis=1)           # row 0: ky=0 taps invalid
    cbot = -TB[:, 2, :].sum(axis=1)
    cleft = -TB[:, :, 0].sum(axis=1)
    cright = -TB[:, :, 2].sum(axis=1)

    bv = np.zeros((128, 9), np.float32)
    bv[0:64, 0] = bias_int;         bv[64:128, 0] = bias_int
    bv[0:64, 1] = bias_int + ctop;  bv[64:128, 1] = bias_int
    bv[0:64, 2] = bias_int;         bv[64:128, 2] = bias_int + cbot
    bv[0:64, 3] = cleft;            bv[64:128, 3] = cleft
    bv[0:64, 4] = cright;           bv[64:128, 4] = cright
    bv[0:64, 5] = TB[:, 0, 0]       # TL corner (+TB[0,0] by inclusion-exclusion)
    bv[0:64, 6] = TB[:, 0, 2]       # TR
    bv[64:128, 7] = TB[:, 2, 0]     # BL
    bv[64:128, 8] = TB[:, 2, 2]     # BR
    return wp, bv


def pack_x(img):
    """Host-side repack of one [64, 256, 256] image into the parity-slot
    layout [129, 128, 258]: slot g holds x rows (2g-1, 2g) — channels of
    row 2g-1 at p=0..63, row 2g at p=64..127 — with 1 zero pad column on
    each side.  Rows -1 and 256 are zeros."""
    xh = np.zeros((129, 2, 64, SLOTW), np.float32)
    # xh[g, 0, c, 1:257] = x[c, 2g-1, :]  (g = 1..128)
    xh[1:, 0, :, 1:W + 1] = img[:, 1::2, :].transpose(1, 0, 2)
    # xh[g, 1, c, 1:257] = x[c, 2g, :]    (g = 0..127)
    xh[:128, 1, :, 1:W + 1] = img[:, 0::2, :].transpose(1, 0, 2)
    import ml_dtypes
    return np.ascontiguousarray(xh.reshape(129, 128, SLOTW)).astype(ml_dtypes.bfloat16)


def unpack_out(od):
    """Inverse of the device output layout [128 pairs, 128 (i c), 256]."""
    return np.ascontiguousarray(
        od.reshape(128, 2, 64, W).transpose(2, 0, 1, 3).reshape(C, H, W)).astype(np.float32)


def build_bass():
    import concourse.bass as bass
    import concourse.bacc as bacc
    import concourse.mybir as mybir
    import concourse.tile as tile

    f32 = mybir.dt.float32
    bf16 = mybir.dt.bfloat16

    nc = bacc.Bacc()
    x = nc.dram_tensor("x", [129, 128, SLOTW], bf16, kind="ExternalInput")
    wp = nc.dram_tensor("wp", [128, 6 * 128], bf16, kind="ExternalInput")
    bvt = nc.dram_tensor("bv", [128, 9], f32, kind="ExternalInput")
    out = nc.dram_tensor("out", [H // 2, 128, W], bf16, kind="ExternalOutput")

    with tile.TileContext(nc) as tc:
        with (
            tc.tile_pool(name="const", bufs=1) as cpool,
            tc.tile_pool(name="xin", bufs=len(STRIPES)) as xpool,
            tc.tile_pool(name="oout", bufs=len(STRIPES)) as opool,
            tc.tile_pool(name="acc", bufs=8, space="PSUM") as ppool,
        ):
            wt = cpool.tile([128, 6, 128], bf16)
            nc.sync.dma_start(out=wt, in_=wp.rearrange("k (w m) -> k w m", w=6))
            bt = cpool.tile([128, 9], f32)
            nc.sync.dma_start(out=bt, in_=bvt[:, :])

            def absorb(engine, aps):
                """Sequencer NoOp reading `aps`: it alone carries the sem
                waits (this walrus allows 1 sync wait per engine/DMA
                instruction; sequencer NoOps may carry several), and later
                instructions on `engine` inherit the observed clock."""
                noop = mybir.InstNoOp(
                    name=nc.get_next_instruction_name(),
                    engine=engine,
                    bass_nofuse=True,
                    ins=[e.lower_ap(ap) for e, ap in aps],
                )
                nc.add_instruction(noop)

            PE, SP, DVE = (mybir.EngineType.PE, mybir.EngineType.SP,
                           mybir.EngineType.DVE)
            # absorb the wt/bt DMA waits once up front
            absorb(PE, [(nc.tensor, wt[:, :, :])])
            absorb(DVE, [(nc.vector, bt[:, :])])

            ot_hist = []
            p0 = 0
            for s, Ss in enumerate(STRIPES):
                NQ = Ss // 2
                last = s == len(STRIPES) - 1
                xt = xpool.tile([128, Ss + 1, SLOTW], bf16, name="xt", tag="xt",
                                padded_shape=[128, S + 1, SLOTW])
                # one 128-partition DMA per stripe: pair-slots incl. halo
                nc.sync.dma_start(
                    out=xt,
                    in_=x[p0:p0 + Ss + 1].rearrange("g p w -> p g w"))

                # absorb the x-DMA wait and (via a read of the stripe-s-2
                # output tile, whose evacuations freed the PSUM banks this
                # stripe reuses) the DVE wait, before any matmul issues
                pe_aps = [(nc.tensor, xt[:, 0:Ss + 1, :])]
                if len(ot_hist) >= 2:
                    pe_aps.append((nc.tensor, ot_hist[-2][:, :, :]))
                absorb(PE, pe_aps)

                # N=512 matmuls: each covers two row-pairs (a "quad"); the
                # rhs is a strided [128, 2, 256] AP over adjacent slots.
                pt = [ppool.tile([128, 2, W], f32, name="pt", tag="pt")
                      for _ in range(NQ)]
                for widx, (ab, kx) in enumerate(PASSES):
                    for q in range(NQ):
                        slot = 2 * q if ab == "A" else 2 * q + 1
                        nc.tensor.matmul(
                            pt[q][:, :, :],
                            lhsT=wt[:, widx, :],
                            rhs=xt[:, slot:slot + 2, kx:kx + W],
                            start=(widx == 0),
                            stop=(widx == len(PASSES) - 1),
                        )

                # bufs=NSTRIPES: ot slots are never recycled, so evacuations
                # never carry a WAR wait on an out-DMA (1-wait budget)
                ot = opool.tile([128, Ss, W], bf16, name="ot", tag="ot",
                                padded_shape=[128, S, W])
                for q in range(NQ):
                    if s == 0 and q == 0:
                        # first pair of the image needs the top-row bias
                        nc.vector.tensor_scalar_add(ot[:, 0, :], pt[q][:, 0, :],
                                                    bt[:, 1:2])
                        nc.vector.tensor_scalar_add(ot[:, 1, :], pt[q][:, 1, :],
                                                    bt[:, 0:1])
                    elif last and q == NQ - 1:
                        nc.vector.tensor_scalar_add(ot[:, Ss - 2, :],
                                                    pt[q][:, 0, :], bt[:, 0:1])
                        nc.vector.tensor_scalar_add(ot[:, Ss - 1, :],
                                                    pt[q][:, 1, :], bt[:, 2:3])
                    else:
                        nc.vector.tensor_scalar_add(
                            ot[:, 2 * q:2 * q + 2, :], pt[q][:, :, :],
                            bt[:, 0:1])

                # column corrections for the bias-through-padding terms
                nc.vector.tensor_scalar_add(ot[:, :, 0:1], ot[:, :, 0:1],
                                            bt[:, 3:4])
                nc.vector.tensor_scalar_add(ot[:, :, W - 1:W], ot[:, :, W - 1:W],
                                            bt[:, 4:5])
                if s == 0:
                    nc.vector.tensor_scalar_add(ot[0:64, 0, 0:1],
                                                ot[0:64, 0, 0:1], bt[0:64, 5:6])
                    nc.vector.tensor_scalar_add(ot[0:64, 0, W - 1:W],
                                                ot[0:64, 0, W - 1:W], bt[0:64, 6:7])
                if last:
                    nc.vector.tensor_scalar_add(ot[64:128, Ss - 1, 0:1],
                                                ot[64:128, Ss - 1, 0:1],
                                                bt[64:128, 7:8])
                    nc.vector.tensor_scalar_add(ot[64:128, Ss - 1, W - 1:W],
                                                ot[64:128, Ss - 1, W - 1:W],
                                                bt[64:128, 8:9])

                absorb(SP, [(nc.sync, ot[:, :, :])])
                nc.sync.dma_start(
                    out=out[p0:p0 + Ss].rearrange("g p w -> p g w"), in_=ot)
                ot_hist.append(ot)
                p0 += Ss

    nc.compile()
    return nc


_BASS_CACHE = {}


def run(inputs, trace=False, tmpdir=None):
    from concourse.bass_utils import run_bass_kernel_spmd

    W_eff, TB = fold_params(
        inputs["w_main"], inputs["bn_main"], inputs["w_1x1"], inputs["bn_1x1"],
        inputs["w_3x3_1"], inputs["bn_3x3_1"], inputs["w_3x3_2"],
        inputs["bn_3x3_2"], inputs["w_avg"], inputs["bn_avg_1"],
        inputs["bn_avg_2"], inputs["c_score"])
    wp, bv = build_device_tensors(W_eff, TB)

    if "nc" not in _BASS_CACHE:
        _BASS_CACHE["nc"] = build_bass()
    nc = _BASS_CACHE["nc"]

    x = np.asarray(inputs["x"], np.float32)
    in_maps = [{"x": pack_x(x[i]), "wp": wp, "bv": bv} for i in range(NCORES)]
    res = run_bass_kernel_spmd(nc, in_maps, core_ids=list(range(NCORES)),
                               trace=trace, tmpdir=tmpdir)
    outs = np.stack([unpack_out(np.asarray(r["out"])) for r in res.results])
    return outs, res


def kernel(**inputs) -> np.ndarray:
    outs, _ = run(inputs, trace=False)
    return outs
